# revision 22
# baseline (speedup 1.0000x reference)
"""Trainium2 Bass kernel for nn_Attention_45930380263558.

EfficientViT-style attention with gathered relative position bias over
x:[16, 1024, 512], data-parallel: 2 batches per core on 8 NeuronCores.

v2 design (vs baseline):
  - Head-PAIR processing: heads (2t, 2t+1) live at partitions 0-63 /
    64-127 of the same qk_sb tile, so their K=64 score matmuls carry
    tile_position (0,0)/(64,0) and run CONCURRENTLY in the PE array
    when emitted back-to-back (row tiling).
  - Rowsum matmuls split into K=64 halves with M=1 outputs at col-group
    partitions {0,32,64,96} of one PSUM bank; each group of 4 rides in
    the shadow of a scores matmul occupying the opposite row-group
    (quadrant concurrency) -> rowsums cost ~no PE slots.
  - One reciprocal per (pair,batch) on the [128,512] rowsum bank
    instead of tiny per-head [2,512] reciprocals.
  - Loop order pair-outer/batch-inner so each pair's 4MB bias table is
    DMA'd once and reused by both batches (halves bias traffic).
  - QKV-form matmuls of the other batch and proj matmuls are emitted as
    FILLER between attention matmul groups, so the PE stays busy while
    ScalarE exp (the true bottleneck, ~168us/core) drains.
  - fp16 output staging (host casts back to fp32).

Precision: fp16 operands for all matmuls, fp32 PSUM accumulation, exp
via exp(SCALE*s)*exp(bias) with a host-precomputed exp-bias table.
Softmax max-subtraction skipped (logits bounded, identical after
normalization).
"""

import os
import sys
from collections import deque

for _p in ("/opt/trn_rl_repo",):
    if _p not in sys.path and os.path.isdir(_p):
        sys.path.insert(0, _p)

from contextlib import ExitStack

import numpy as np

import concourse.bass as bass
import concourse.tile as tile
from concourse import bacc, mybir
from concourse.bass_utils import run_bass_kernel_spmd

F32 = mybir.dt.float32
F16 = mybir.dt.float16

N_CORES = 8
B = 16
B_LOC = B // N_CORES  # 2
N = 1024  # tokens
D = 512  # model dim
H = 8  # heads
DK = 64  # key dim
DV = 128  # value dim per head
SCALE = DK ** -0.5
NT = N // 128  # 8 token tiles
DC = D // 128  # 4 dim chunks
QH = 2  # q halves of 512
NPAIR = H // 2  # 4 head pairs
LAG_RS = 2  # kc lag of rowsums behind scores
LAG_PV = 3  # kc lag of PV (extra slack for the o-ring epilogue chain)

LAST_RESULT = None


def _ensure_axon_hooks_module():
    try:
        import antenv.axon_hooks  # noqa: F401
        return
    except ImportError:
        pass
    import types

    import antenv

    m = types.ModuleType("antenv.axon_hooks")
    m._hook = None

    def set_axon_ntff_profile_hook(h):
        m._hook = h

    def get_axon_ntff_profile_hook():
        return m._hook

    m.set_axon_ntff_profile_hook = set_axon_ntff_profile_hook
    m.get_axon_ntff_profile_hook = get_axon_ntff_profile_hook
    sys.modules["antenv.axon_hooks"] = m
    antenv.axon_hooks = m


_ensure_axon_hooks_module()


def build_program(use_qkv_bias: bool, use_proj_bias: bool):
    nc = bacc.Bacc("TRN2", target_bir_lowering=False, debug=False,
                   num_devices=N_CORES)

    xT_d = nc.dram_tensor("xT", [B_LOC, DC, 128, N], F16, kind="ExternalInput").ap()
    w_qk_d = nc.dram_tensor("w_qk", [DC, 128, N], F16, kind="ExternalInput").ap()
    w_v_d = nc.dram_tensor("w_v", [DC, 128, N], F16, kind="ExternalInput").ap()
    bias_d = nc.dram_tensor("bias", [H, NT, 128, N], F16, kind="ExternalInput").ap()
    w_proj_d = nc.dram_tensor("w_proj", [H, 128, D], F16, kind="ExternalInput").ap()
    ones_d = nc.dram_tensor("ones", [128, N], F16, kind="ExternalInput").ap()
    selb_d = nc.dram_tensor("selb", [128, 4, 128], F16, kind="ExternalInput").ap()
    inv_scr = nc.dram_tensor("inv_scratch", [B_LOC, H, N], F16).ap()
    out_d = nc.dram_tensor("out", [B_LOC, N, D], F16, kind="ExternalOutput").ap()
    if use_qkv_bias:
        qk_bias_d = nc.dram_tensor("qk_bias", [1, N], F16, kind="ExternalInput").ap()
        v_bias_d = nc.dram_tensor("v_bias", [1, N], F16, kind="ExternalInput").ap()
    if use_proj_bias:
        proj_bias_d = nc.dram_tensor("proj_bias", [1, D], F16, kind="ExternalInput").ap()

    with tile.TileContext(nc) as tc, ExitStack() as ctx:
        consts = ctx.enter_context(tc.tile_pool(name="consts", bufs=1))
        # bigp holds both batches' x^T tiles AND the bias ring; all tiles
        # are [128, 4, N] f16 (1 MiB).  FIFO aliasing evicts x after the
        # b1 forms drain, then rotates over bias halves pair by pair.
        bigp = ctx.enter_context(tc.tile_pool(name="bigp", bufs=7))
        qkp = ctx.enter_context(tc.tile_pool(name="qkp", bufs=1))
        vp = ctx.enter_context(tc.tile_pool(name="vp", bufs=1))
        ptp = ctx.enter_context(tc.tile_pool(name="ptp", bufs=8))
        invp = ctx.enter_context(tc.tile_pool(name="invp", bufs=1))
        bcp = ctx.enter_context(tc.tile_pool(name="bcp", bufs=2))
        onp = ctx.enter_context(tc.tile_pool(name="onp", bufs=1))
        outp = ctx.enter_context(tc.tile_pool(name="outp", bufs=2))

        # PSUM: s 3x[128,512]=3 banks, o 2x[128,1024]=4 banks, rs 1 bank
        ps_s = ctx.enter_context(tc.tile_pool(name="ps_s", bufs=3, space="PSUM"))
        ps_o = ctx.enter_context(tc.tile_pool(name="ps_o", bufs=2, space="PSUM"))
        ps_rs = ctx.enter_context(tc.tile_pool(name="ps_rs", bufs=1, space="PSUM"))

        # ---- constants (x first so the first form matmuls start early) ----
        x_t = {}
        x_t[0] = bigp.tile([128, DC, N], F16, name="x_0", tag="big")
        w_qk_t = consts.tile([128, DC, N], F16)
        for kc in range(DC):
            nc.sync.dma_start(out=x_t[0][:, kc, :], in_=xT_d[0, kc])
            nc.sync.dma_start(out=w_qk_t[:, kc, :], in_=w_qk_d[kc])
        w_v_t = consts.tile([128, DC, N], F16)
        for kc in range(DC):
            nc.sync.dma_start(out=w_v_t[:, kc, :], in_=w_v_d[kc])
        ones_t = consts.tile([128, N], F16)
        nc.sync.dma_start(out=ones_t, in_=ones_d)
        selb_t = consts.tile([128, 4, 128], F16)
        nc.sync.dma_start(out=selb_t, in_=selb_d)
        x_t[1] = bigp.tile([128, DC, N], F16, name="x_1", tag="big")
        for kc in range(DC):
            nc.sync.dma_start(out=x_t[1][:, kc, :], in_=xT_d[1, kc])
        w_proj_t = consts.tile([128, H, D], F16)
        nc.sync.dma_start(out=w_proj_t, in_=w_proj_d.transpose([1, 0, 2]))
        ones_col_top = ones_t[0:64, 0:1]
        ones_col_bot = ones_t[64:128, 0:1]
        ones_row = ones_t[0:1, 0:128]
        if use_qkv_bias:
            qk_bias_t = consts.tile([1, N], F16)
            nc.sync.dma_start(out=qk_bias_t, in_=qk_bias_d)
            v_bias_t = consts.tile([1, N], F16)
            nc.sync.dma_start(out=v_bias_t, in_=v_bias_d)
            ones_n = ones_t[0:1, :]
        if use_proj_bias:
            proj_bias_t = consts.tile([1, D], F16)
            nc.sync.dma_start(out=proj_bias_t, in_=proj_bias_d)

        qk_sb = {b: qkp.tile([128, NT, N], F16, name=f"qk_sb_{b}") for b in range(B_LOC)}
        v_sb = {b: vp.tile([128, NT, N], F16, name=f"v_sb_{b}") for b in range(B_LOC)}
        on8 = {b: onp.tile([128, H, N], F16, name=f"on8_{b}") for b in range(B_LOC)}

        # ---- form / proj units ----
        def form1_unit(b, mt, nt):
            def emit():
                w_col = w_qk_t[:, :, mt * 128:(mt + 1) * 128]
                qp = ps_s.tile([128, 512], F32, tag="s")
                for kc in range(DC):
                    nc.tensor.matmul(
                        qp,
                        lhsT=w_col[:, kc, :],
                        rhs=x_t[b][:, kc, nt * 512:(nt + 1) * 512],
                        start=(kc == 0),
                        stop=(kc == DC - 1 and not use_qkv_bias),
                    )
                if use_qkv_bias:
                    nc.tensor.matmul(
                        qp,
                        lhsT=qk_bias_t[:, mt * 128:(mt + 1) * 128],
                        rhs=ones_n[:, nt * 512:(nt + 1) * 512],
                        start=False, stop=True,
                    )
                with nc.allow_low_precision(reason="fp16 activations"):
                    nc.vector.tensor_copy(
                        qk_sb[b][:, mt, nt * 512:(nt + 1) * 512], qp)
            return emit

        def form2_unit(b, tt, nt):
            def emit():
                qp = ps_s.tile([128, 512], F32, tag="s")
                for kc in range(DC):
                    nc.tensor.matmul(
                        qp,
                        lhsT=x_t[b][:, kc, tt * 128:(tt + 1) * 128],
                        rhs=w_v_t[:, kc, nt * 512:(nt + 1) * 512],
                        start=(kc == 0),
                        stop=(kc == DC - 1 and not use_qkv_bias),
                    )
                if use_qkv_bias:
                    nc.tensor.matmul(
                        qp,
                        lhsT=ones_n[:, tt * 128:(tt + 1) * 128],
                        rhs=v_bias_t[:, nt * 512:(nt + 1) * 512],
                        start=False, stop=True,
                    )
                with nc.allow_low_precision(reason="fp16 activations"):
                    nc.vector.tensor_copy(
                        v_sb[b][:, tt, nt * 512:(nt + 1) * 512], qp)
            return emit

        def proj_unit(b, qt):
            def emit():
                pp = ps_s.tile([128, D], F32, tag="s")
                for h in range(H):
                    last = (h == H - 1)
                    nc.tensor.matmul(
                        pp,
                        lhsT=on8[b][:, h, qt * 128:(qt + 1) * 128],
                        rhs=w_proj_t[:, h, :],
                        start=(h == 0),
                        stop=(last and not use_proj_bias),
                    )
                if use_proj_bias:
                    nc.tensor.matmul(
                        pp,
                        lhsT=ones_row,
                        rhs=proj_bias_t,
                        start=False, stop=True,
                    )
                ot = outp.tile([128, D], F16)
                with nc.allow_low_precision(reason="fp16 output"):
                    nc.vector.tensor_copy(ot, pp)
                nc.sync.dma_start(
                    out=out_d[b, qt * 128:(qt + 1) * 128, :], in_=ot)
            return emit

        fillers = deque()

        def emit_filler(k):
            n = 0
            while fillers and n < k:
                fillers.popleft()()
                n += 1

        # ---- prologue: Form(b0) inline; Form(b1) queued as filler ----
        for mt in range(NT):
            for nt in range(QH):
                form1_unit(0, mt, nt)()
        for tt in range(NT):
            for nt in range(QH):
                form2_unit(0, tt, nt)()

        # b1 forms ordered by earliest need: pair0 q/k tiles first
        for mt in (0, 4):
            for nt in range(QH):
                fillers.append(form1_unit(1, mt, nt))
        for tt in range(4):
            for nt in range(QH):
                fillers.append(form2_unit(1, tt, nt))
        for mt in (1, 5):
            for nt in range(QH):
                fillers.append(form1_unit(1, mt, nt))
        for tt in range(4, NT):
            for nt in range(QH):
                fillers.append(form2_unit(1, tt, nt))
        late_forms = {1: [], 2: []}
        for mt in (2, 6):
            for nt in range(QH):
                late_forms[1].append(form1_unit(1, mt, nt))
        for mt in (3, 7):
            for nt in range(QH):
                late_forms[2].append(form1_unit(1, mt, nt))

        # ---- bias ring: tile per (pair, hh, half) = [128, 4, N] ----
        bias_tiles = {}

        def load_bias(t, hh):
            h = 2 * t + hh
            for half in range(2):
                bt = bigp.tile([128, 4, N], F16, name=f"bias_{t}_{hh}_{half}", tag="big")
                nc.sync.dma_start(
                    out=bt,
                    in_=bias_d[h].transpose([1, 0, 2])[:, half * 4:half * 4 + 4, :],
                )
                bias_tiles[(t, hh, half)] = bt

        load_bias(0, 0)
        load_bias(0, 1)

        # ---- attention combos ----
        state = {"pending_epi": None}

        def emit_epilogue(b, t, o_pair, rs_ps):
            inv_t = invp.tile([128, 512], F16)
            with nc.allow_low_precision(reason="softmax denominators"):
                nc.vector.reciprocal(inv_t, rs_ps)
            for hh in range(2):
                h = 2 * t + hh
                for qh in range(QH):
                    c = 32 * (2 * hh + qh)
                    nc.sync.dma_start(
                        out=inv_scr[b, h, qh * 512:(qh + 1) * 512]
                        .rearrange("(a c) -> a c", a=1),
                        in_=inv_t[c:c + 1, :])
            for hh in range(2):
                h = 2 * t + hh
                bc_t = bcp.tile([128, N], F16)
                bcast_src = bass.AP(
                    tensor=inv_scr.tensor,
                    offset=inv_scr.offset + (b * H + h) * N,
                    ap=[[0, 128], [1, N]],
                )
                nc.sync.dma_start(out=bc_t, in_=bcast_src)
                with nc.allow_low_precision(reason="fp16 normalized O^T"):
                    nc.vector.tensor_tensor(on8[b][:, h, :], o_pair[hh], bc_t,
                                            op=mybir.AluOpType.mult)
            # once the LAST pair of a batch is normalized, its proj units
            # become emission-safe
            if t == NPAIR - 1:
                for qt in range(NT):
                    fillers.append(proj_unit(b, qt))

        for t in range(NPAIR):
            for bi in range(B_LOC):
                if bi == 0 and t in late_forms:
                    fillers.extend(late_forms.pop(t))
                # prefetch next pair's bias (staggered halves)
                if bi == 0 and t + 1 < NPAIR:
                    load_bias(t + 1, 0)
                elif bi == 1 and t + 1 < NPAIR:
                    load_bias(t + 1, 1)

                qT = {hh: qk_sb[bi][64 * hh:64 * hh + 64, t, :] for hh in range(2)}
                kT = {hh: qk_sb[bi][64 * hh:64 * hh + 64, 4 + t, :] for hh in range(2)}

                o_pair = {hh: ps_o.tile([128, N], F32, tag="o", name=f"o_{t}_{bi}_{hh}") for hh in range(2)}
                rs_ps = ps_rs.tile([128, 512], F32, tag="rs")

                pt_hist = {}  # kc -> {hh: pt_tile}

                def emit_rs_half(kcl, bot, pts, rs_ps=rs_ps):
                    # safe full-K rowsums: block one-hot stationary puts
                    # each (hh, qh) colsum in its own 32-row strip of the
                    # rs bank; two per call (bot -> hh split)
                    hh = 1 if bot else 0
                    for qh in range(QH):
                        s = 2 * hh + qh
                        nc.tensor.matmul(
                            rs_ps,
                            lhsT=selb_t[:, s, :],
                            rhs=pts[hh][:, qh * 512:(qh + 1) * 512],
                            start=(kcl == 0 and qh == 0 and bot),
                            stop=(kcl == NT - 1 and qh == QH - 1 and not bot),
                            skip_group_check=True,
                        )

                def emit_pv(kcl, pts, o_pair=o_pair, bi=bi, t=t):
                    for hh in range(2):
                        ph = 2 * t + hh
                        for qh in range(QH):
                            nc.tensor.matmul(
                                o_pair[hh][:, qh * 512:(qh + 1) * 512],
                                lhsT=v_sb[bi][:, kcl, ph * 128:(ph + 1) * 128],
                                rhs=pts[hh][:, qh * 512:(qh + 1) * 512],
                                start=(kcl == 0), stop=(kcl == NT - 1),
                            )

                for kc in range(NT + LAG_PV):
                    has_sc = kc < NT
                    kcl2 = kc - LAG_RS if 0 <= kc - LAG_RS < NT else None
                    kcl3 = kc - LAG_PV if kc - LAG_PV >= 0 else None

                    if has_sc:
                        pts = {}
                        pts[0] = ptp.tile([128, N], F16, name=f"pt0_{kc}", tag="pt")
                        pts[1] = ptp.tile([128, N], F16, name=f"pt1_{kc}", tag="pt")
                        pt_hist[kc] = pts
                        sA = ps_s.tile([128, 512], F32, tag="s")
                        sB = ps_s.tile([128, 512], F32, tag="s")
                        # slot 1: both heads' qh0 scores
                        nc.tensor.matmul(
                            sA, lhsT=kT[0][:, kc * 128:(kc + 1) * 128],
                            rhs=qT[0][:, 0:512], start=True, stop=True)
                        nc.tensor.matmul(
                            sB, lhsT=kT[1][:, kc * 128:(kc + 1) * 128],
                            rhs=qT[1][:, 0:512], start=True, stop=True)
                        with nc.allow_low_precision(reason="fp16 exp"):
                            nc.scalar.activation(
                                pts[0][:, 0:512], sA,
                                mybir.ActivationFunctionType.Exp, scale=SCALE)
                            nc.scalar.activation(
                                pts[1][:, 0:512], sB,
                                mybir.ActivationFunctionType.Exp, scale=SCALE)
                    emit_filler(1)

                    if has_sc:
                        # slot 2: h0 qh1 scores
                        sC = ps_s.tile([128, 512], F32, tag="s")
                        nc.tensor.matmul(
                            sC, lhsT=kT[0][:, kc * 128:(kc + 1) * 128],
                            rhs=qT[0][:, 512:1024], start=True, stop=True)
                    if kcl2 is not None:
                        emit_rs_half(kcl2, True, pt_hist[kcl2])
                    if has_sc:
                        with nc.allow_low_precision(reason="fp16 exp"):
                            nc.scalar.activation(
                                pts[0][:, 512:1024], sC,
                                mybir.ActivationFunctionType.Exp, scale=SCALE)

                        # slot 3: h1 qh1 scores
                        sD = ps_s.tile([128, 512], F32, tag="s")
                        nc.tensor.matmul(
                            sD, lhsT=kT[1][:, kc * 128:(kc + 1) * 128],
                            rhs=qT[1][:, 512:1024], start=True, stop=True)
                    if kcl2 is not None:
                        emit_rs_half(kcl2, False, pt_hist[kcl2])
                    if has_sc:
                        with nc.allow_low_precision(reason="fp16 exp"):
                            nc.scalar.activation(
                                pts[1][:, 512:1024], sD,
                                mybir.ActivationFunctionType.Exp, scale=SCALE)

                        # bias multiplies, in place (exp output lives in pt)
                        for hh in range(2):
                            bt = bias_tiles[(t, hh, kc // 4)]
                            with nc.allow_low_precision(reason="fp16 P^T"):
                                nc.vector.tensor_tensor(
                                    pts[hh], pts[hh], bt[:, kc % 4, :],
                                    op=mybir.AluOpType.mult)

                    # deferred epilogue of the previous combo
                    if kc == 1 and state["pending_epi"] is not None:
                        emit_epilogue(*state["pending_epi"])
                        state["pending_epi"] = None

                    if kcl3 is not None:
                        emit_pv(kcl3, pt_hist[kcl3])
                        del pt_hist[kcl3]
                    emit_filler(1)

                state["pending_epi"] = (bi, t, o_pair, rs_ps)

        emit_epilogue(*state["pending_epi"])
        emit_filler(len(fillers) + 1)

    nc.compile()
    return nc


def _prep_core_inputs(x, qkv_w, qkv_b, proj_w, proj_b, attn_biases, bias_idxs):
    """Host-side layout preparation. Returns (shared, per_core_xT, flags)."""
    x = np.ascontiguousarray(np.asarray(x, np.float32))
    qkv_w = np.asarray(qkv_w, np.float32)
    qkv_b = np.asarray(qkv_b, np.float32)
    proj_w = np.asarray(proj_w, np.float32)
    proj_b = np.asarray(proj_b, np.float32)
    attn_biases = np.asarray(attn_biases, np.float32)
    bias_idxs = np.asarray(bias_idxs)

    Wh = qkv_w.reshape(D, H, 256)
    w_q = Wh[:, :, :DK].reshape(D, H * DK)
    w_k = Wh[:, :, DK:2 * DK].reshape(D, H * DK)
    w_qk = np.concatenate([w_q, w_k], axis=1)
    w_v = Wh[:, :, 2 * DK:].reshape(D, H * DV)

    bh = qkv_b.reshape(H, 256)
    qk_bias = np.concatenate([bh[:, :DK].reshape(-1), bh[:, DK:2 * DK].reshape(-1)])
    v_bias = bh[:, 2 * DK:].reshape(-1)

    BT = np.ascontiguousarray(
        np.exp(attn_biases[:, bias_idxs]).transpose(0, 2, 1))
    bias = BT.reshape(H, NT, 128, N).astype(np.float16)

    selb = np.zeros((128, 4, 128), np.float16)
    for s in range(4):
        selb[:, s, s * 32:(s + 1) * 32] = 1.0
    shared = {
        "ones": np.ones((128, N), np.float16),
        "selb": selb,
        "w_qk": np.ascontiguousarray(w_qk.reshape(DC, 128, H * DK * 2)).astype(np.float16),
        "w_v": np.ascontiguousarray(w_v.reshape(DC, 128, H * DV)).astype(np.float16),
        "bias": bias,
        "w_proj": np.ascontiguousarray(proj_w.reshape(H, 128, D)).astype(np.float16),
    }
    use_qkv_bias = bool(np.any(qkv_b))
    use_proj_bias = bool(np.any(proj_b))
    if use_qkv_bias:
        shared["qk_bias"] = qk_bias.reshape(1, N).astype(np.float16)
        shared["v_bias"] = v_bias.reshape(1, N).astype(np.float16)
    if use_proj_bias:
        shared["proj_bias"] = proj_b.reshape(1, D).astype(np.float16)

    xT = np.ascontiguousarray(x.transpose(0, 2, 1)).reshape(B, DC, 128, N)
    xT = xT.astype(np.float16)
    per_core = [xT[c * B_LOC:(c + 1) * B_LOC] for c in range(N_CORES)]
    return shared, per_core, use_qkv_bias, use_proj_bias


def kernel(x, qkv_w, qkv_b, proj_w, proj_b, attn_biases, bias_idxs):
    global LAST_RESULT
    shared, per_core, use_qkv_bias, use_proj_bias = _prep_core_inputs(
        x, qkv_w, qkv_b, proj_w, proj_b, attn_biases, bias_idxs)

    nc = build_program(use_qkv_bias, use_proj_bias)

    in_maps = [dict(shared, xT=per_core[c]) for c in range(N_CORES)]
    trace = bool(os.environ.get("BASS_TRACE"))
    res = run_bass_kernel_spmd(nc, in_maps, core_ids=list(range(N_CORES)),
                               trace=trace)
    LAST_RESULT = res
    out = np.concatenate([res.results[c]["out"] for c in range(N_CORES)], axis=0)
    return np.ascontiguousarray(out.astype(np.float32))


# revision 23
# speedup vs baseline: 1.1967x; 1.1967x over previous
"""Trainium2 Bass kernel for nn_Attention_45930380263558.

EfficientViT-style attention with gathered relative position bias over
x:[16, 1024, 512], data-parallel: 2 batches per core on 8 NeuronCores.

v2 design (vs baseline):
  - Head-PAIR processing: heads (2t, 2t+1) live at partitions 0-63 /
    64-127 of the same qk_sb tile, so their K=64 score matmuls carry
    tile_position (0,0)/(64,0) and run CONCURRENTLY in the PE array
    when emitted back-to-back (row tiling).
  - Rowsum matmuls split into K=64 halves with M=1 outputs at col-group
    partitions {0,32,64,96} of one PSUM bank; each group of 4 rides in
    the shadow of a scores matmul occupying the opposite row-group
    (quadrant concurrency) -> rowsums cost ~no PE slots.
  - One reciprocal per (pair,batch) on the [128,512] rowsum bank
    instead of tiny per-head [2,512] reciprocals.
  - Loop order pair-outer/batch-inner so each pair's 4MB bias table is
    DMA'd once and reused by both batches (halves bias traffic).
  - QKV-form matmuls of the other batch and proj matmuls are emitted as
    FILLER between attention matmul groups, so the PE stays busy while
    ScalarE exp (the true bottleneck, ~168us/core) drains.
  - fp16 output staging (host casts back to fp32).

Precision: fp16 operands for all matmuls, fp32 PSUM accumulation, exp
via exp(SCALE*s)*exp(bias) with a host-precomputed exp-bias table.
Softmax max-subtraction skipped (logits bounded, identical after
normalization).
"""

import os
import sys
from collections import deque

for _p in ("/opt/trn_rl_repo",):
    if _p not in sys.path and os.path.isdir(_p):
        sys.path.insert(0, _p)

from contextlib import ExitStack

import numpy as np

import concourse.bass as bass
import concourse.tile as tile
from concourse import bacc, mybir
from concourse.bass_utils import run_bass_kernel_spmd

F32 = mybir.dt.float32
F16 = mybir.dt.float16

N_CORES = 8
B = 16
B_LOC = B // N_CORES  # 2
N = 1024  # tokens
D = 512  # model dim
H = 8  # heads
DK = 64  # key dim
DV = 128  # value dim per head
SCALE = DK ** -0.5
NT = N // 128  # 8 token tiles
DC = D // 128  # 4 dim chunks
QH = 2  # q halves of 512
NPAIR = H // 2  # 4 head pairs
LAG = 2  # kc lag of pt-consumers (PV / rowsum) behind scores

LAST_RESULT = None


def _ensure_axon_hooks_module():
    try:
        import antenv.axon_hooks  # noqa: F401
        return
    except ImportError:
        pass
    import types

    import antenv

    m = types.ModuleType("antenv.axon_hooks")
    m._hook = None

    def set_axon_ntff_profile_hook(h):
        m._hook = h

    def get_axon_ntff_profile_hook():
        return m._hook

    m.set_axon_ntff_profile_hook = set_axon_ntff_profile_hook
    m.get_axon_ntff_profile_hook = get_axon_ntff_profile_hook
    sys.modules["antenv.axon_hooks"] = m
    antenv.axon_hooks = m


_ensure_axon_hooks_module()


def build_program(use_qkv_bias: bool, use_proj_bias: bool):
    nc = bacc.Bacc("TRN2", target_bir_lowering=False, debug=False,
                   num_devices=N_CORES)

    xT_d = nc.dram_tensor("xT", [B_LOC, DC, 128, N], F16, kind="ExternalInput").ap()
    w_qk_d = nc.dram_tensor("w_qk", [DC, 128, N], F16, kind="ExternalInput").ap()
    w_v_d = nc.dram_tensor("w_v", [DC, 128, N], F16, kind="ExternalInput").ap()
    bias_d = nc.dram_tensor("bias", [H, NT, 128, N], F16, kind="ExternalInput").ap()
    w_proj_d = nc.dram_tensor("w_proj", [H, 128, D], F16, kind="ExternalInput").ap()
    ones_d = nc.dram_tensor("ones", [128, N], F16, kind="ExternalInput").ap()
    selb_d = nc.dram_tensor("selb", [128, 4, 128], F16, kind="ExternalInput").ap()
    inv_scr = nc.dram_tensor("inv_scratch", [B_LOC, H, N], F16).ap()
    out_d = nc.dram_tensor("out", [B_LOC, N, D], F16, kind="ExternalOutput").ap()
    if use_qkv_bias:
        qk_bias_d = nc.dram_tensor("qk_bias", [1, N], F16, kind="ExternalInput").ap()
        v_bias_d = nc.dram_tensor("v_bias", [1, N], F16, kind="ExternalInput").ap()
    if use_proj_bias:
        proj_bias_d = nc.dram_tensor("proj_bias", [1, D], F16, kind="ExternalInput").ap()

    with tile.TileContext(nc) as tc, ExitStack() as ctx:
        consts = ctx.enter_context(tc.tile_pool(name="consts", bufs=1))
        # bigp holds both batches' x^T tiles AND the bias ring; all tiles
        # are [128, 4, N] f16 (1 MiB).  FIFO aliasing evicts x after the
        # b1 forms drain, then rotates over bias halves pair by pair.
        bigp = ctx.enter_context(tc.tile_pool(name="bigp", bufs=7))
        qkp = ctx.enter_context(tc.tile_pool(name="qkp", bufs=1))
        vp = ctx.enter_context(tc.tile_pool(name="vp", bufs=1))
        ep = ctx.enter_context(tc.tile_pool(name="ep", bufs=3))
        ptp = ctx.enter_context(tc.tile_pool(name="ptp", bufs=4))
        invp = ctx.enter_context(tc.tile_pool(name="invp", bufs=1))
        bcp = ctx.enter_context(tc.tile_pool(name="bcp", bufs=2))
        onp = ctx.enter_context(tc.tile_pool(name="onp", bufs=1))
        outp = ctx.enter_context(tc.tile_pool(name="outp", bufs=2))

        # PSUM: s 3x[128,512]=3 banks, o 2x[128,1024]=4 banks, rs 1 bank
        ps_s = ctx.enter_context(tc.tile_pool(name="ps_s", bufs=3, space="PSUM"))
        ps_o = ctx.enter_context(tc.tile_pool(name="ps_o", bufs=2, space="PSUM"))
        ps_rs = ctx.enter_context(tc.tile_pool(name="ps_rs", bufs=1, space="PSUM"))

        # ---- constants (x first so the first form matmuls start early) ----
        x_t = {}
        x_t[0] = bigp.tile([128, DC, N], F16, name="x_0", tag="big")
        w_qk_t = consts.tile([128, DC, N], F16)
        for kc in range(DC):
            nc.sync.dma_start(out=x_t[0][:, kc, :], in_=xT_d[0, kc])
            nc.sync.dma_start(out=w_qk_t[:, kc, :], in_=w_qk_d[kc])
        w_v_t = consts.tile([128, DC, N], F16)
        for kc in range(DC):
            nc.sync.dma_start(out=w_v_t[:, kc, :], in_=w_v_d[kc])
        ones_t = consts.tile([128, N], F16)
        nc.sync.dma_start(out=ones_t, in_=ones_d)
        selb_t = consts.tile([128, 4, 128], F16)
        nc.sync.dma_start(out=selb_t, in_=selb_d)
        x_t[1] = bigp.tile([128, DC, N], F16, name="x_1", tag="big")
        for kc in range(DC):
            nc.sync.dma_start(out=x_t[1][:, kc, :], in_=xT_d[1, kc])
        w_proj_t = consts.tile([128, H, D], F16)
        nc.sync.dma_start(out=w_proj_t, in_=w_proj_d.transpose([1, 0, 2]))
        ones_col_top = ones_t[0:64, 0:1]
        ones_col_bot = ones_t[64:128, 0:1]
        ones_row = ones_t[0:1, 0:128]
        if use_qkv_bias:
            qk_bias_t = consts.tile([1, N], F16)
            nc.sync.dma_start(out=qk_bias_t, in_=qk_bias_d)
            v_bias_t = consts.tile([1, N], F16)
            nc.sync.dma_start(out=v_bias_t, in_=v_bias_d)
            ones_n = ones_t[0:1, :]
        if use_proj_bias:
            proj_bias_t = consts.tile([1, D], F16)
            nc.sync.dma_start(out=proj_bias_t, in_=proj_bias_d)

        qk_sb = {b: qkp.tile([128, NT, N], F16, name=f"qk_sb_{b}") for b in range(B_LOC)}
        v_sb = {b: vp.tile([128, NT, N], F16, name=f"v_sb_{b}") for b in range(B_LOC)}
        on8 = {b: onp.tile([128, H, N], F16, name=f"on8_{b}") for b in range(B_LOC)}

        # ---- form / proj units ----
        def form1_unit(b, mt, nt):
            def emit():
                w_col = w_qk_t[:, :, mt * 128:(mt + 1) * 128]
                qp = ps_s.tile([128, 512], F32, tag="s")
                for kc in range(DC):
                    nc.tensor.matmul(
                        qp,
                        lhsT=w_col[:, kc, :],
                        rhs=x_t[b][:, kc, nt * 512:(nt + 1) * 512],
                        start=(kc == 0),
                        stop=(kc == DC - 1 and not use_qkv_bias),
                    )
                if use_qkv_bias:
                    nc.tensor.matmul(
                        qp,
                        lhsT=qk_bias_t[:, mt * 128:(mt + 1) * 128],
                        rhs=ones_n[:, nt * 512:(nt + 1) * 512],
                        start=False, stop=True,
                    )
                with nc.allow_low_precision(reason="fp16 activations"):
                    nc.vector.tensor_copy(
                        qk_sb[b][:, mt, nt * 512:(nt + 1) * 512], qp)
            return emit

        def form2_unit(b, tt, nt):
            def emit():
                qp = ps_s.tile([128, 512], F32, tag="s")
                for kc in range(DC):
                    nc.tensor.matmul(
                        qp,
                        lhsT=x_t[b][:, kc, tt * 128:(tt + 1) * 128],
                        rhs=w_v_t[:, kc, nt * 512:(nt + 1) * 512],
                        start=(kc == 0),
                        stop=(kc == DC - 1 and not use_qkv_bias),
                    )
                if use_qkv_bias:
                    nc.tensor.matmul(
                        qp,
                        lhsT=ones_n[:, tt * 128:(tt + 1) * 128],
                        rhs=v_bias_t[:, nt * 512:(nt + 1) * 512],
                        start=False, stop=True,
                    )
                with nc.allow_low_precision(reason="fp16 activations"):
                    nc.vector.tensor_copy(
                        v_sb[b][:, tt, nt * 512:(nt + 1) * 512], qp)
            return emit

        def proj_unit(b, qt):
            def emit():
                pp = ps_s.tile([128, D], F32, tag="s")
                for h in range(H):
                    last = (h == H - 1)
                    nc.tensor.matmul(
                        pp,
                        lhsT=on8[b][:, h, qt * 128:(qt + 1) * 128],
                        rhs=w_proj_t[:, h, :],
                        start=(h == 0),
                        stop=(last and not use_proj_bias),
                    )
                if use_proj_bias:
                    nc.tensor.matmul(
                        pp,
                        lhsT=ones_row,
                        rhs=proj_bias_t,
                        start=False, stop=True,
                    )
                ot = outp.tile([128, D], F16)
                with nc.allow_low_precision(reason="fp16 output"):
                    nc.vector.tensor_copy(ot, pp)
                nc.sync.dma_start(
                    out=out_d[b, qt * 128:(qt + 1) * 128, :], in_=ot)
            return emit

        fillers = deque()

        def emit_filler(k):
            n = 0
            while fillers and n < k:
                fillers.popleft()()
                n += 1

        # ---- prologue: Form(b0) inline; Form(b1) queued as filler ----
        for mt in range(NT):
            for nt in range(QH):
                form1_unit(0, mt, nt)()
        for tt in range(NT):
            for nt in range(QH):
                form2_unit(0, tt, nt)()

        # b1 forms ordered by earliest need: pair0 q/k tiles first
        for mt in (0, 4):
            for nt in range(QH):
                fillers.append(form1_unit(1, mt, nt))
        for tt in range(4):
            for nt in range(QH):
                fillers.append(form2_unit(1, tt, nt))
        for mt in (1, 5):
            for nt in range(QH):
                fillers.append(form1_unit(1, mt, nt))
        for tt in range(4, NT):
            for nt in range(QH):
                fillers.append(form2_unit(1, tt, nt))
        late_forms = {1: [], 2: []}
        for mt in (2, 6):
            for nt in range(QH):
                late_forms[1].append(form1_unit(1, mt, nt))
        for mt in (3, 7):
            for nt in range(QH):
                late_forms[2].append(form1_unit(1, mt, nt))

        # ---- bias ring: tile per (pair, hh, half) = [128, 4, N] ----
        bias_tiles = {}

        def load_bias(t, hh):
            h = 2 * t + hh
            for half in range(2):
                bt = bigp.tile([128, 4, N], F16, name=f"bias_{t}_{hh}_{half}", tag="big")
                nc.sync.dma_start(
                    out=bt,
                    in_=bias_d[h].transpose([1, 0, 2])[:, half * 4:half * 4 + 4, :],
                )
                bias_tiles[(t, hh, half)] = bt

        load_bias(0, 0)
        load_bias(0, 1)

        # ---- attention combos ----
        state = {"pending_epi": None}

        def emit_epilogue(b, t, o_pair, rs_ps):
            inv_t = invp.tile([128, 512], F16)
            with nc.allow_low_precision(reason="softmax denominators"):
                nc.vector.reciprocal(inv_t, rs_ps)
            for hh in range(2):
                h = 2 * t + hh
                for qh in range(QH):
                    c = 32 * (2 * hh + qh)
                    nc.sync.dma_start(
                        out=inv_scr[b, h, qh * 512:(qh + 1) * 512]
                        .rearrange("(a c) -> a c", a=1),
                        in_=inv_t[c:c + 1, :])
            for hh in range(2):
                h = 2 * t + hh
                bc_t = bcp.tile([128, N], F16)
                bcast_src = bass.AP(
                    tensor=inv_scr.tensor,
                    offset=inv_scr.offset + (b * H + h) * N,
                    ap=[[0, 128], [1, N]],
                )
                nc.sync.dma_start(out=bc_t, in_=bcast_src)
                with nc.allow_low_precision(reason="fp16 normalized O^T"):
                    nc.vector.tensor_tensor(on8[b][:, h, :], o_pair[hh], bc_t,
                                            op=mybir.AluOpType.mult)
            # once the LAST pair of a batch is normalized, its proj units
            # become emission-safe
            if t == NPAIR - 1:
                for qt in range(NT):
                    fillers.append(proj_unit(b, qt))

        for t in range(NPAIR):
            for bi in range(B_LOC):
                if bi == 0 and t in late_forms:
                    fillers.extend(late_forms.pop(t))
                # prefetch next pair's bias (staggered halves)
                if bi == 0 and t + 1 < NPAIR:
                    load_bias(t + 1, 0)
                elif bi == 1 and t + 1 < NPAIR:
                    load_bias(t + 1, 1)

                qT = {hh: qk_sb[bi][64 * hh:64 * hh + 64, t, :] for hh in range(2)}
                kT = {hh: qk_sb[bi][64 * hh:64 * hh + 64, 4 + t, :] for hh in range(2)}

                o_pair = {hh: ps_o.tile([128, N], F32, tag="o", name=f"o_{t}_{bi}_{hh}") for hh in range(2)}
                rs_ps = ps_rs.tile([128, 512], F32, tag="rs")

                pt_hist = deque()  # (kc, {hh: pt_tile})

                def emit_rs_half(kcl, bot, pts, rs_ps=rs_ps):
                    # safe full-K rowsums: block one-hot stationary puts
                    # each (hh, qh) colsum in its own 32-row strip of the
                    # rs bank; two per call (bot -> hh split)
                    hh = 1 if bot else 0
                    for qh in range(QH):
                        s = 2 * hh + qh
                        nc.tensor.matmul(
                            rs_ps,
                            lhsT=selb_t[:, s, :],
                            rhs=pts[hh][:, qh * 512:(qh + 1) * 512],
                            start=(kcl == 0 and qh == 0 and bot),
                            stop=(kcl == NT - 1 and qh == QH - 1 and not bot),
                            skip_group_check=True,
                        )

                def emit_pv(kcl, pts, o_pair=o_pair, bi=bi, t=t):
                    for hh in range(2):
                        ph = 2 * t + hh
                        for qh in range(QH):
                            nc.tensor.matmul(
                                o_pair[hh][:, qh * 512:(qh + 1) * 512],
                                lhsT=v_sb[bi][:, kcl, ph * 128:(ph + 1) * 128],
                                rhs=pts[hh][:, qh * 512:(qh + 1) * 512],
                                start=(kcl == 0), stop=(kcl == NT - 1),
                            )

                for kc in range(NT + LAG):
                    has_sc = kc < NT
                    kcl = kc - LAG
                    lag_pts = None
                    if kcl >= 0:
                        lag_pts = pt_hist.popleft()[1]

                    e_tiles = {}
                    if has_sc:
                        sA = ps_s.tile([128, 512], F32, tag="s")
                        sB = ps_s.tile([128, 512], F32, tag="s")
                        # slot 1: both heads' qh0 scores, concurrent rows
                        nc.tensor.matmul(
                            sA, lhsT=kT[0][:, kc * 128:(kc + 1) * 128],
                            rhs=qT[0][:, 0:512], start=True, stop=True)
                        nc.tensor.matmul(
                            sB, lhsT=kT[1][:, kc * 128:(kc + 1) * 128],
                            rhs=qT[1][:, 0:512], start=True, stop=True)
                        e_tiles[0] = ep.tile([128, N], F16, name=f"e0_{t}_{bi}_{kc}", tag="e")
                        e_tiles[1] = ep.tile([128, N], F16, name=f"e1_{t}_{bi}_{kc}", tag="e")
                        with nc.allow_low_precision(reason="fp16 exp"):
                            nc.scalar.activation(
                                e_tiles[0][:, 0:512], sA,
                                mybir.ActivationFunctionType.Exp, scale=SCALE)
                            nc.scalar.activation(
                                e_tiles[1][:, 0:512], sB,
                                mybir.ActivationFunctionType.Exp, scale=SCALE)
                    emit_filler(1)

                    if has_sc:
                        # slot 2: h0 qh1 scores (rows 0-63) || rs bottoms
                        sC = ps_s.tile([128, 512], F32, tag="s")
                        nc.tensor.matmul(
                            sC, lhsT=kT[0][:, kc * 128:(kc + 1) * 128],
                            rhs=qT[0][:, 512:1024], start=True, stop=True)
                    if lag_pts is not None:
                        emit_rs_half(kcl, True, lag_pts)
                    if has_sc:
                        with nc.allow_low_precision(reason="fp16 exp"):
                            nc.scalar.activation(
                                e_tiles[0][:, 512:1024], sC,
                                mybir.ActivationFunctionType.Exp, scale=SCALE)

                        # slot 3: h1 qh1 scores (rows 64-127) || rs tops
                        sD = ps_s.tile([128, 512], F32, tag="s")
                        nc.tensor.matmul(
                            sD, lhsT=kT[1][:, kc * 128:(kc + 1) * 128],
                            rhs=qT[1][:, 512:1024], start=True, stop=True)
                    if lag_pts is not None:
                        emit_rs_half(kcl, False, lag_pts)
                    if has_sc:
                        with nc.allow_low_precision(reason="fp16 exp"):
                            nc.scalar.activation(
                                e_tiles[1][:, 512:1024], sD,
                                mybir.ActivationFunctionType.Exp, scale=SCALE)

                        # bias multiplies -> pt tiles
                        pts = {}
                        for hh in range(2):
                            pt_t = ptp.tile([128, N], F16)
                            bt = bias_tiles[(t, hh, kc // 4)]
                            with nc.allow_low_precision(reason="fp16 P^T"):
                                nc.vector.tensor_tensor(
                                    pt_t, e_tiles[hh], bt[:, kc % 4, :],
                                    op=mybir.AluOpType.mult)
                            pts[hh] = pt_t
                        pt_hist.append((kc, pts))

                    # deferred epilogue of the previous combo
                    if kc == 1 and state["pending_epi"] is not None:
                        emit_epilogue(*state["pending_epi"])
                        state["pending_epi"] = None

                    if lag_pts is not None:
                        emit_pv(kcl, lag_pts)
                    emit_filler(1)

                state["pending_epi"] = (bi, t, o_pair, rs_ps)

        emit_epilogue(*state["pending_epi"])
        emit_filler(len(fillers) + 1)

    nc.compile()
    return nc


def _prep_core_inputs(x, qkv_w, qkv_b, proj_w, proj_b, attn_biases, bias_idxs):
    """Host-side layout preparation. Returns (shared, per_core_xT, flags)."""
    x = np.ascontiguousarray(np.asarray(x, np.float32))
    qkv_w = np.asarray(qkv_w, np.float32)
    qkv_b = np.asarray(qkv_b, np.float32)
    proj_w = np.asarray(proj_w, np.float32)
    proj_b = np.asarray(proj_b, np.float32)
    attn_biases = np.asarray(attn_biases, np.float32)
    bias_idxs = np.asarray(bias_idxs)

    Wh = qkv_w.reshape(D, H, 256)
    w_q = Wh[:, :, :DK].reshape(D, H * DK)
    w_k = Wh[:, :, DK:2 * DK].reshape(D, H * DK)
    w_qk = np.concatenate([w_q, w_k], axis=1)
    w_v = Wh[:, :, 2 * DK:].reshape(D, H * DV)

    bh = qkv_b.reshape(H, 256)
    qk_bias = np.concatenate([bh[:, :DK].reshape(-1), bh[:, DK:2 * DK].reshape(-1)])
    v_bias = bh[:, 2 * DK:].reshape(-1)

    BT = np.ascontiguousarray(
        np.exp(attn_biases[:, bias_idxs]).transpose(0, 2, 1))
    bias = BT.reshape(H, NT, 128, N).astype(np.float16)

    selb = np.zeros((128, 4, 128), np.float16)
    for s in range(4):
        selb[:, s, s * 32:(s + 1) * 32] = 1.0
    shared = {
        "ones": np.ones((128, N), np.float16),
        "selb": selb,
        "w_qk": np.ascontiguousarray(w_qk.reshape(DC, 128, H * DK * 2)).astype(np.float16),
        "w_v": np.ascontiguousarray(w_v.reshape(DC, 128, H * DV)).astype(np.float16),
        "bias": bias,
        "w_proj": np.ascontiguousarray(proj_w.reshape(H, 128, D)).astype(np.float16),
    }
    use_qkv_bias = bool(np.any(qkv_b))
    use_proj_bias = bool(np.any(proj_b))
    if use_qkv_bias:
        shared["qk_bias"] = qk_bias.reshape(1, N).astype(np.float16)
        shared["v_bias"] = v_bias.reshape(1, N).astype(np.float16)
    if use_proj_bias:
        shared["proj_bias"] = proj_b.reshape(1, D).astype(np.float16)

    xT = np.ascontiguousarray(x.transpose(0, 2, 1)).reshape(B, DC, 128, N)
    xT = xT.astype(np.float16)
    per_core = [xT[c * B_LOC:(c + 1) * B_LOC] for c in range(N_CORES)]
    return shared, per_core, use_qkv_bias, use_proj_bias


def kernel(x, qkv_w, qkv_b, proj_w, proj_b, attn_biases, bias_idxs):
    global LAST_RESULT
    shared, per_core, use_qkv_bias, use_proj_bias = _prep_core_inputs(
        x, qkv_w, qkv_b, proj_w, proj_b, attn_biases, bias_idxs)

    nc = build_program(use_qkv_bias, use_proj_bias)

    in_maps = [dict(shared, xT=per_core[c]) for c in range(N_CORES)]
    trace = bool(os.environ.get("BASS_TRACE"))
    res = run_bass_kernel_spmd(nc, in_maps, core_ids=list(range(N_CORES)),
                               trace=trace)
    LAST_RESULT = res
    out = np.concatenate([res.results[c]["out"] for c in range(N_CORES)], axis=0)
    return np.ascontiguousarray(out.astype(np.float32))
